# revision 46
# baseline (speedup 1.0000x reference)
"""Channel-self-attention (LayerNorm + grouped-1x1-qkv + channel softmax attn
+ residual) on 8 TRN2 NeuronCores.

v2: pair-sharding (core r = batch r//2, spatial half r%2), restructured
(244us -> ~147us typical, best 137us):
 - host pre-scales the transposed A/K shard by gamma and ships rhs_m2
   ([g*x_V; 1; -g; b], padded to 128 partitions: <128-partition DMAs stick
   to ONE DMA engine at ~26 GB/s; 128-partition DMAs fan over all 16)
 - ALL big loads ride the sync engine's hardware DMA queue in chunk order
   (x-ctile1, x-ctile0, xst quarter) x4 so Scalar Square-stats, DVE
   bn_stats and the PE Gram all chase the stream; AllReduce triggers ~60us
 - Sx for ch 0..170 comes free from a 1/g column in the Gram (ptk row 3 /
   col 89); Scalar only does Square passes, DVE does ctile1 via bn_stats
 - ONE pairwise AllReduce of [ptk 90x90 | stats 128x4] (~28us incl. ~11.5us
   fixed trigger->start latency)
 - post-AR logits algebra folded into matmul contraction rows (rank-1 terms
   ride extra lhsT/rhs rows; rA/rK applied as per-partition scales on DVE),
   ACT tables (sqrt) preloaded pre-AR via a dummy op
 - c1/c2 rows of the V-application folded into the ntc matmul via a dynamic
   w0 column (w0 @ (rV*mV)) and a host-const column (w_v); NT split per
   output half so it=0 m2 matmuls overlap it=1 softmax
 - softmax recip folded into att (bf16); epilogue residual-add: 26 tiles on
   DVE (tensor_tensor from PSUM), 6 via identity-matmul accumulate + Scalar
   copy; stores alternate sync/scalar hardware queues
"""
import sys

sys.path.insert(0, "/opt/trn_rl_repo")

import numpy as np
import ml_dtypes

B, C = 4, 256
S = 32 * 32 * 32          # 32768
NCORES = 8
SHH = S // 2              # 16384 per-core spatial half
NST = SHH // 128          # 128 stiles
EPS = 1e-5
SCALE = float(S) ** -0.5
NCH = 4                   # x load chunks per ctile
CHW = SHH // NCH          # 4096 cols per chunk
UTW2 = 176                # xst columns: A(86) + [g, b, 1, 1/g] + K(86)

_BF = ml_dtypes.bfloat16

_cache = {}


def _build_program():
    from contextlib import ExitStack
    import concourse.bass as bass
    import concourse.bacc as bacc
    import concourse.tile as tile
    from concourse import mybir, masks

    f32 = mybir.dt.float32
    bf16 = mybir.dt.bfloat16
    AF = mybir.ActivationFunctionType
    OP = mybir.AluOpType
    AX = mybir.AxisListType

    nc = bacc.Bacc(
        "TRN2",
        target_bir_lowering=False,
        debug=False,
        enable_asserts=False,
        num_devices=NCORES,
    )

    RG = [[0, 1], [2, 3], [4, 5], [6, 7]]

    # ---------------- DRAM I/O ----------------
    xs_d = nc.dram_tensor("xs", [C, SHH], bf16, kind="ExternalInput")
    xst_d = nc.dram_tensor("xst", [128, NST * UTW2], bf16, kind="ExternalInput")
    rhsm_d = nc.dram_tensor("rhsm", [128, SHH], bf16, kind="ExternalInput")
    ekt_d = nc.dram_tensor("ekt", [86, C], f32, kind="ExternalInput")
    eqtL_d = nc.dram_tensor("eqtL", [88, C], f32, kind="ExternalInput")
    w0e_d = nc.dram_tensor("w0e", [2 * 128, 89], bf16, kind="ExternalInput")
    w0t_d = nc.dram_tensor("w0t", [86, C], bf16, kind="ExternalInput")
    crows_d = nc.dram_tensor("crows", [4, C], f32, kind="ExternalInput")
    idb_d = nc.dram_tensor("idb", [128, 128], bf16, kind="ExternalInput")
    sc_d = nc.dram_tensor("sc", [1, 8], f32, kind="ExternalInput")
    out_d = nc.dram_tensor("out", [C, SHH], bf16, kind="ExternalOutput")

    # AllReduce bounce layout: [ptk 128*90 | stats p-major 128*4].
    # The Gram lhsT is padded to 128 columns so the ptk bounce write is a
    # 128-partition DMA (fans over all 16 engines; a 90-partition write
    # sticks to one engine at ~6 GB/s). Real ptk rows start at 38.
    PB = 128 * 90
    ST_OFF = PB
    TOT = PB + 512
    PR0 = 38

    with tile.TileContext(nc) as tc, ExitStack() as ctx:
        const = ctx.enter_context(tc.tile_pool(name="const", bufs=1))
        xpool = ctx.enter_context(tc.tile_pool(name="xpool", bufs=1))
        rhsp = ctx.enter_context(tc.tile_pool(name="rhsp", bufs=1))
        small = ctx.enter_context(tc.tile_pool(name="small", bufs=2))
        dram = ctx.enter_context(tc.tile_pool(name="dram", bufs=1, space="DRAM"))

        # ------------- early CC bootstrap -------------
        # The NRT collective bootstrap barrier attaches to the FIRST CC op.
        # A dummy 8-float AllReduce up front moves the 8-core rendezvous to
        # ~5us (during loads) so core-start skew cannot gate the real
        # AllReduce at ~55us.
        bncd_in = dram.tile([8], f32, tag="bncdi")
        bncd_out = dram.tile([8], f32, tag="bncdo")
        nc.gpsimd.dma_start(out=bncd_in[:], in_=sc_d[0:1, :].opt())
        nc.gpsimd.collective_compute(
            "AllReduce", OP.add, replica_groups=RG,
            ins=[bncd_in[:].opt()], outs=[bncd_out[:].opt()])

        # ------------- constants (gpsimd queue) -------------
        ident = const.tile([128, 128], f32)
        masks.make_identity(nc, ident[:])
        identb = const.tile([128, 128], bf16)
        nc.gpsimd.dma_start(out=identb[:], in_=idb_d.ap())
        ekt_sb = const.tile([86, C], f32)
        nc.gpsimd.dma_start(out=ekt_sb[:], in_=ekt_d.ap())
        eqtL_sb = const.tile([88, C], f32)
        nc.gpsimd.dma_start(out=eqtL_sb[:], in_=eqtL_d.ap())
        w0e_sb = const.tile([128, 2, 89], bf16)
        for jt in range(2):
            nc.gpsimd.dma_start(out=w0e_sb[:, jt, :],
                                in_=w0e_d[jt * 128:(jt + 1) * 128, :])
        w0t_sb = const.tile([86, C], bf16)
        nc.gpsimd.dma_start(out=w0t_sb[:], in_=w0t_d.ap())
        crows_sb = const.tile([4, C], f32)
        nc.gpsimd.dma_start(out=crows_sb[:], in_=crows_d.ap())
        sc_bc = const.tile([128, 8], f32)
        nc.gpsimd.dma_start(
            out=sc_bc[:],
            in_=bass.AP(tensor=sc_d, offset=0, ap=[[0, 128], [1, 8]]))

        # rhs_u const rows 88 (wk), 89 (bk) preloaded
        rhs_u = const.tile([90, C], f32)
        nc.gpsimd.dma_start(out=rhs_u[88:90, :], in_=crows_d[0:2, :])
        cr2_sb = const.tile([2, C], f32)
        nc.gpsimd.dma_start(out=cr2_sb[:], in_=crows_d[2:4, :])

        # ------------- big loads -------------
        # ALL big loads ride the sync engine's hardware DMA queue (one hw
        # queue sustains ~330 GB/s; the gpsimd SWDGE queue crawls). The sync
        # engine does nothing else, so enqueue ring-full stalls are free.
        # Interleave ctile0/ctile1 chunks so both stats engines chase.
        x_sb = xpool.tile([128, 2, NCH, 8, 512], bf16)
        ut_sb = xpool.tile([128, NST, UTW2], bf16)
        # 128 partitions: transfers with <128 partitions stick to ONE DMA
        # engine (~26 GB/s); 128-partition transfers fan over all 16.
        rhs_m2 = rhsp.tile([128, SHH], bf16)
        # x chunks first (stats chains run concurrently with the stream and
        # finish by ~38us); xst LAST in 8 fine pieces so the post-last-byte
        # tail is one 16-stile Gram chase (~1.4us), minimizing the AllReduce
        # trigger time; rhsm (needed only at ~100us) trails.
        NQ8 = NST // 8
        for chk in range(NCH):
            nc.sync.dma_start(
                out=x_sb[:, 1, chk, :, :],
                in_=xs_d[128:256, chk * CHW:(chk + 1) * CHW])
            nc.sync.dma_start(
                out=x_sb[:, 0, chk, :, :],
                in_=xs_d[0:128, chk * CHW:(chk + 1) * CHW])
        for q8 in range(8):
            nc.sync.dma_start(
                out=ut_sb[:, NQ8 * q8:NQ8 * (q8 + 1), :],
                in_=xst_d[:, NQ8 * q8 * UTW2:NQ8 * (q8 + 1) * UTW2])

        # ------------- stats (chase chunks) -------------
        # Scalar: Square-only on ctile0 (Sx for ch 0..127 comes from the 1/g
        # Gram row/col). DVE: bn_stats on ctile1.
        scratch = const.tile([128, NCH, 8, 512], bf16)
        sq0a = const.tile([128, NCH], f32)
        bno = const.tile([128, NCH, 8, 6], f32)
        for chk in range(NCH):
            nc.scalar.activation(
                out=scratch[:, chk, :, :], in_=x_sb[:, 0, chk, :, :],
                func=AF.Square, accum_out=sq0a[:, chk:chk + 1])
            for g in range(8):
                nc.vector.bn_stats(out=bno[:, chk, g, :],
                                   in_=x_sb[:, 1, chk, g, :])

        stats4 = const.tile([128, 2, 2], f32)   # (0, Sxx0, Sx1, Sxx1)
        # dummy Sqrt: pulls the sqrt ACT table load into the pre-AR window
        dum = small.tile([1, 1], f32, tag="dum", bufs=1)
        nc.vector.memset(dum[:], 1.0)
        nc.scalar.activation(out=dum[:], in_=dum[:], func=AF.Sqrt)
        nc.vector.memset(stats4[:, 0, 0:1], 0.0)
        nc.vector.reduce_sum(stats4[:, 0, 1:2], sq0a[:], axis=AX.X)
        mv1 = const.tile([128, 2], f32)
        nc.vector.bn_aggr(out=mv1[:], in_=bno[:])
        nc.vector.tensor_scalar_mul(stats4[:, 1, 0:1], mv1[:, 0:1], float(SHH))
        nc.vector.scalar_tensor_tensor(
            out=stats4[:, 1, 1:2], in0=mv1[:, 0:1], scalar=mv1[:, 0:1],
            in1=mv1[:, 1:2], op0=OP.mult, op1=OP.add)
        nc.vector.tensor_scalar_mul(stats4[:, 1, 1:2], stats4[:, 1, 1:2],
                                    float(SHH))

        bnc_in = dram.tile([TOT], f32)
        bnc_out = dram.tile([TOT], f32)
        nc.scalar.dma_start(
            out=bnc_in[ST_OFF:ST_OFF + 512].rearrange("(p k) -> p k", k=4),
            in_=stats4[:])

        # ------------- Gram (chase quarters) -------------
        with tc.tile_pool(name="s1ps", bufs=1, space="PSUM") as stg1ps:
            ptk_ps = stg1ps.tile([128, 90], f32)
            for st in range(NST):
                nc.tensor.matmul(
                    ptk_ps[:], lhsT=ut_sb[:, st, 48:176],
                    rhs=ut_sb[:, st, 0:90],
                    start=(st == 0), stop=(st == NST - 1))
            ptk_sb = small.tile([128, 90], f32, tag="ptksb", bufs=1)
            nc.vector.tensor_copy(ptk_sb[:], ptk_ps[:])
            nc.scalar.dma_start(
                out=bnc_in[0:PB].rearrange("(p f) -> p f", f=90),
                in_=ptk_sb[:])
            for chk in range(4):
                nc.scalar.dma_start(
                    out=rhs_m2[:, chk * 4096:(chk + 1) * 4096],
                    in_=rhsm_d[:, chk * 4096:(chk + 1) * 4096])

        nc.gpsimd.collective_compute(
            "AllReduce", OP.add,
            replica_groups=RG,
            ins=[bnc_in[:].opt()], outs=[bnc_out[:].opt()])

        # ------------- post-AR readbacks -------------
        # ptk rows: 0=g,1=b,2=1,3=1/g,4..89=K; cols 0..85=A, 86..89=g,b,1,1/g
        # lhsT_u: parts 0..85 = ptk rows 4..89 (cols 86..88 = tK/gK/hK),
        # parts 86..89 filled by pack-transpose.
        lhsT_u = const.tile([128, 90], f32)
        nc.sync.dma_start(
            out=lhsT_u[0:86, 0:89],
            in_=bass.AP(tensor=bnc_out.tensor,
                        offset=bnc_out.offset + (PR0 + 4) * 90,
                        ap=[[90, 86], [1, 89]]))
        st4b = const.tile([128, 2, 2], f32)
        nc.sync.dma_start(
            out=st4b[:],
            in_=bass.AP(tensor=bnc_out.tensor,
                        offset=bnc_out.offset + ST_OFF,
                        ap=[[4, 128], [1, 4]]))
        # raw Sx for ctile0: ch 0..85 from the 1/g row, ch 86..127 from the
        # 1/g column (K rows 5..46)
        sx0 = small.tile([128, 1], f32, tag="sx0", bufs=1)
        nc.sync.dma_start(
            out=sx0[0:86, :],
            in_=bass.AP(tensor=bnc_out.tensor,
                        offset=bnc_out.offset + (PR0 + 3) * 90,
                        ap=[[1, 86], [1, 1]]))
        nc.sync.dma_start(
            out=sx0[86:128, :],
            in_=bass.AP(tensor=bnc_out.tensor,
                        offset=bnc_out.offset + (PR0 + 5) * 90 + 89,
                        ap=[[90, 42], [1, 1]]))
        pack = small.tile([86, 4], f32, tag="pack", bufs=1)
        nc.sync.dma_start(
            out=pack[:, 1:2],
            in_=bass.AP(tensor=bnc_out.tensor,
                        offset=bnc_out.offset + PR0 * 90,
                        ap=[[1, 86], [1, 1]]))
        ua_col = small.tile([86, 2], f32, tag="uahacol", bufs=1)
        nc.sync.dma_start(
            out=ua_col[:, 0:1],
            in_=bass.AP(tensor=bnc_out.tensor,
                        offset=bnc_out.offset + (PR0 + 1) * 90,
                        ap=[[1, 86], [1, 1]]))
        nc.sync.dma_start(
            out=ua_col[:, 1:2],
            in_=bass.AP(tensor=bnc_out.tensor,
                        offset=bnc_out.offset + (PR0 + 2) * 90,
                        ap=[[1, 86], [1, 1]]))

        # ------------- native-layout mean/var -------------
        mnat = small.tile([128, 2], f32, tag="mnat", bufs=1)
        nc.vector.tensor_scalar_mul(mnat[:, 0:1], sx0[:], 1.0 / S)
        nc.vector.tensor_scalar_mul(mnat[:, 1:2], st4b[:, 1, 0:1], 1.0 / S)
        vnat = small.tile([128, 2], f32, tag="vnat", bufs=1)
        nc.vector.tensor_scalar(
            out=vnat[:], in0=st4b[:, :, 1], scalar1=1.0 / S, scalar2=EPS,
            op0=OP.mult, op1=OP.add)
        msq = small.tile([128, 2], f32, tag="msq", bufs=1)
        nc.vector.tensor_mul(msq[:], mnat[:], mnat[:])
        nc.vector.tensor_sub(vnat[:], vnat[:], msq[:])
        nc.scalar.activation(out=vnat[:], in_=vnat[:], func=AF.Sqrt)
        rnat = small.tile([128, 2], f32, tag="rnat", bufs=1)
        nc.vector.reciprocal(rnat[:], vnat[:])

        # splices: K spans ctile0 p85..127 + ctile1 p0..42; V = ctile1 p42..127
        mrk = small.tile([86, 4], f32, tag="mrk", bufs=1)   # mK, rK, mV, rV
        nc.sync.dma_start(out=mrk[0:43, 0:1], in_=mnat[85:128, 0:1])
        nc.sync.dma_start(out=mrk[43:86, 0:1], in_=mnat[0:43, 1:2])
        nc.scalar.dma_start(out=mrk[0:43, 1:2], in_=rnat[85:128, 0:1])
        nc.scalar.dma_start(out=mrk[43:86, 1:2], in_=rnat[0:43, 1:2])
        nc.scalar.dma_start(out=mrk[:, 2:3], in_=mnat[42:128, 1:2])
        nc.sync.dma_start(out=mrk[:, 3:4], in_=rnat[42:128, 1:2])
        mK, rK = mrk[:, 0:1], mrk[:, 1:2]
        mV, rV = mrk[:, 2:3], mrk[:, 3:4]
        mA, rA = mnat[0:86, 0:1], rnat[0:86, 0:1]
        tK = lhsT_u[0:86, 86:87]
        gK = lhsT_u[0:86, 87:88]
        hK = lhsT_u[0:86, 88:89]
        scG2 = sc_bc[0:86, 1:2]
        scNGb = sc_bc[0:86, 4:5]
        scNG1 = sc_bc[0:86, 5:6]

        # ------------- auxL / pack chains -------------
        # auxL column order: 0 = syK (q3), 1 = t3c (q2), 2 = q0, 3 = q1 so
        # that aux_ps rows 0:2 are the const-add rows (base-0 accesses).
        auxL = small.tile([86, 4], f32, tag="auxL", bufs=1)
        # col2: q0 = rK*(G2*mK - tK)
        nc.vector.scalar_tensor_tensor(
            out=auxL[:, 2:3], in0=mK, scalar=scG2, in1=tK,
            op0=OP.mult, op1=OP.subtract)
        nc.vector.tensor_mul(auxL[:, 2:3], auxL[:, 2:3], rK)
        # col3: q1 = -rK*mK
        nc.vector.scalar_tensor_tensor(
            out=auxL[:, 3:4], in0=mK, scalar=-1.0, in1=rK,
            op0=OP.mult, op1=OP.mult)
        # col1: t3c = rK*(gK - Gb*mK)
        nc.vector.scalar_tensor_tensor(
            out=auxL[:, 1:2], in0=mK, scalar=scNGb, in1=gK,
            op0=OP.mult, op1=OP.add)
        nc.vector.tensor_mul(auxL[:, 1:2], auxL[:, 1:2], rK)
        # col0: syK = rK*(hK - G1*mK) + B1
        nc.vector.scalar_tensor_tensor(
            out=auxL[:, 0:1], in0=mK, scalar=scNG1, in1=hK,
            op0=OP.mult, op1=OP.add)
        nc.vector.tensor_mul(auxL[:, 0:1], auxL[:, 0:1], rK)
        nc.vector.tensor_scalar(
            out=auxL[:, 0:1], in0=auxL[:, 0:1], scalar1=sc_bc[0:86, 3:4],
            scalar2=None, op0=OP.add)

        # pack cols: 0 = mA, 1 = tA (DMA'd), 2 = uA - Gb*mA, 3 = hA - G1*mA
        nc.vector.tensor_copy(pack[:, 0:1], mA)
        nc.vector.scalar_tensor_tensor(
            out=pack[:, 2:3], in0=mA, scalar=scNGb, in1=ua_col[:, 0:1],
            op0=OP.mult, op1=OP.add)
        nc.vector.scalar_tensor_tensor(
            out=pack[:, 3:4], in0=mA, scalar=scNG1, in1=ua_col[:, 1:2],
            op0=OP.mult, op1=OP.add)

        att_n = []
        recip2 = small.tile([128, 2], f32, tag="recip2", bufs=1)
        z2 = small.tile([128, 2], f32, tag="z2", bufs=1)
        rv_ext = small.tile([128, 1], f32, tag="rvext", bufs=1)
        u_sb = small.tile([88, C], f32, tag="usb", bufs=1)
        lhs_m2 = small.tile([89, C], bf16, tag="lhsm2", bufs=1)

        with tc.tile_pool(name="psA", bufs=2, space="PSUM") as psA, \
             tc.tile_pool(name="psB", bufs=2, space="PSUM") as psB:
            packT_ps = psA.tile([4, 86], f32, tag="psA", name="pT")
            nc.tensor.transpose(packT_ps[:], pack[:], ident[0:86, 0:86])
            packT_sb = small.tile([4, 86], f32, tag="packTsb", bufs=1)
            nc.vector.tensor_copy(packT_sb[:], packT_ps[:])
            nc.scalar.dma_start(out=lhsT_u[86:90, 0:86], in_=packT_sb[:])

            # ------------- aux / u / logits matmuls -------------
            # aux_ps rows: 0 = syK-row (R), 1 = t3c-row, 2 = q0, 3 = q1
            aux_ps = psA.tile([4, C], f32, tag="psA", name="aux")
            nc.tensor.matmul(aux_ps[:], lhsT=auxL[:], rhs=ekt_sb[:],
                             start=True, stop=True)
            nc.vector.tensor_scalar(
                out=rhs_u[0:86, :], in0=ekt_sb[:], scalar1=rK, scalar2=None,
                op0=OP.mult)
            aux_sb = small.tile([4, C], f32, tag="auxsb", bufs=1)
            nc.vector.tensor_copy(aux_sb[:], aux_ps[:])
            nc.scalar.dma_start(out=rhs_u[86:88, :], in_=aux_sb[2:4, :])

            # uex2: row0 = R = syK-row + S*bk, row1 = t3c-row + (B1*bk+B2*wk)
            uex2 = small.tile([2, C], f32, tag="uex2", bufs=1)
            nc.vector.tensor_add(uex2[:], aux_ps[0:2, :], cr2_sb[:])
            nc.scalar.dma_start(out=u_sb[86:88, :], in_=uex2[:])

            u_ps = psB.tile([86, C], f32, tag="psB", name="ups")
            nc.tensor.matmul(u_ps[:], lhsT=lhsT_u[0:90, 0:86], rhs=rhs_u[:],
                             start=True, stop=True)
            nc.vector.tensor_scalar(
                out=u_sb[0:86, :], in0=u_ps[:], scalar1=rA, scalar2=None,
                op0=OP.mult)

            # w0c (c1 fold) — off critical path, needs only rV*mV
            rvmv = small.tile([86, 1], bf16, tag="rvmv", bufs=1)
            nc.vector.scalar_tensor_tensor(
                out=rvmv[:], in0=mV, scalar=1.0, in1=rV,
                op0=OP.mult, op1=OP.mult)
            w0c_ps = psA.tile([128, 2], f32, tag="psA", name="w0c")
            for jt in range(2):
                nc.tensor.matmul(
                    w0c_ps[:, jt:jt + 1],
                    lhsT=w0t_sb[:, jt * 128:(jt + 1) * 128], rhs=rvmv[:],
                    start=True, stop=True)
                nc.vector.tensor_copy(w0e_sb[:, jt, 87:88],
                                      w0c_ps[:, jt:jt + 1])

            nc.vector.memset(rv_ext[64:128, :], 1.0)
            nc.vector.tensor_copy(rv_ext[0:64, :], mrk[0:64, 3:4])
            # rows 64..85 must still be rV — rewrite them via a 32-aligned op
            nc.vector.tensor_copy(rv_ext[64:86, :], mrk[64:86, 3:4])

            # ------------- softmax -------------
            for it in range(2):
                log_ps = psB.tile([128, C], f32, tag="psB", name=f"lg{it}")
                nc.tensor.matmul(
                    log_ps[:], lhsT=eqtL_sb[:, it * 128:(it + 1) * 128],
                    rhs=u_sb[:], start=True, stop=True)
                rmax = small.tile([128, 1], f32, tag="rmax", name=f"rm{it}")
                nc.vector.reduce_max(rmax[:], log_ps[:], axis=AX.X)
                nbias = small.tile([128, 1], f32, tag="nbias", name=f"nb{it}")
                nc.vector.tensor_scalar_mul(nbias[:], rmax[:], -SCALE)
                a_bf = small.tile([128, C], bf16, tag=f"abf{it}",
                                  name=f"ab{it}")
                nc.scalar.activation(
                    out=a_bf[:], in_=log_ps[:], func=AF.Exp,
                    bias=nbias[:], scale=SCALE, accum_out=z2[:, it:it + 1])
                nc.vector.reciprocal(recip2[:, it:it + 1], z2[:, it:it + 1])
                an = small.tile([128, C], bf16, tag=f"attn{it}",
                                name=f"an{it}")
                nc.vector.tensor_scalar(
                    out=an[:], in0=a_bf[:], scalar1=recip2[:, it:it + 1],
                    scalar2=None, op0=OP.mult)
                att_n.append(an)

        # ------------- NT: ntc = w0e^T @ att_n^T, per output half -------
        # Split by output half (it) so the it=0 m2 matmuls can start while
        # it=1 softmax/transposes are still in flight.
        with tc.tile_pool(name="psC", bufs=4, space="PSUM") as psC:
            for it in range(2):
                ntc_ps = psC.tile([89, 128], f32, tag="psCn", name=f"ntc{it}")
                at_ps = psC.tile([128, C], bf16, tag="psCa", name=f"atp{it}")
                for jt in range(2):
                    nc.tensor.transpose(
                        at_ps[:, jt * 128:(jt + 1) * 128],
                        att_n[it][:, jt * 128:(jt + 1) * 128],
                        identb[:])
                at_bf = small.tile([128, C], bf16, tag=f"atbf{it}",
                                   name=f"atb{it}")
                nc.scalar.copy(at_bf[:], at_ps[:])
                for jt in range(2):
                    nc.tensor.matmul(
                        ntc_ps[:], lhsT=w0e_sb[:, jt, :],
                        rhs=at_bf[:, jt * 128:(jt + 1) * 128],
                        start=(jt == 0), stop=(jt == 1))
                nc.scalar.activation(
                    out=lhs_m2[:, it * 128:(it + 1) * 128], in_=ntc_ps[:],
                    func=AF.Copy, scale=rv_ext[0:89, :])

        # ------------- m2 + epilogue -------------
        osml = ctx.enter_context(tc.tile_pool(name="osml", bufs=6))
        psD = ctx.enter_context(tc.tile_pool(name="psD", bufs=4, space="PSUM"))
        for it in range(2):
            for ch in range(16):
                ostg = osml.tile([128, 2, 512], bf16, tag="ostg", bufs=6,
                                 name=f"o{it}{ch}")
                o_ps = psD.tile([128, 2, 512], f32, tag="psD", bufs=4,
                                name=f"op{it}{ch}")
                ident_tile = (ch in (5, 11, 15))
                for h in range(2):
                    if ident_tile:
                        # residual via PE: PSUM = x, then accumulate att@V
                        nc.tensor.matmul(
                            o_ps[:, h, :], lhsT=identb[:],
                            rhs=x_sb[:, it, ch // 4,
                                     (ch % 4) * 2 + h, :],
                            start=True, stop=False)
                    nc.tensor.matmul(
                        o_ps[:, h, :],
                        lhsT=lhs_m2[0:89, it * 128:(it + 1) * 128],
                        rhs=rhs_m2[0:89,
                                   (2 * ch + h) * 512:(2 * ch + h + 1) * 512],
                        start=not ident_tile, stop=True)
                if ident_tile:
                    nc.scalar.copy(ostg[:], o_ps[:])
                else:
                    xr2 = x_sb[:, it, ch // 4,
                               (ch % 4) * 2:(ch % 4) * 2 + 2, :]
                    nc.vector.tensor_tensor(
                        out=ostg[:], in0=o_ps[:], in1=xr2, op=OP.add)
                deng = nc.sync if (ch % 2 == 0) else nc.scalar
                deng.dma_start(
                    out=out_d[it * 128:(it + 1) * 128,
                              ch * 1024:(ch + 1) * 1024],
                    in_=ostg[:])

    nc.compile()
    return nc


def _host_prep(x, gamma, beta, w_qkv, b_qkv):
    xf = np.ascontiguousarray(np.asarray(x, np.float32).reshape(B * C, S))
    gam = np.asarray(gamma, np.float32).reshape(-1)
    bet = np.asarray(beta, np.float32).reshape(-1)
    w_qkv = np.asarray(w_qkv, np.float32)
    b_qkv = np.asarray(b_qkv, np.float32)
    w_q, w_k, w_v = w_qkv[:C], w_qkv[C:2 * C], w_qkv[2 * C:]
    b_q, b_k, b_v = b_qkv[:C], b_qkv[C:2 * C], b_qkv[2 * C:]

    ii = np.arange(C)
    ekt = np.zeros((86, C), np.float32)
    ekt[(C + ii) // 3 - 85, ii] = w_k
    eqtL = np.zeros((88, C), np.float32)
    eqtL[ii // 3, ii] = w_q
    eqtL[86] = b_q
    eqtL[87] = w_q
    w0 = np.zeros((C, 87), np.float32)
    w0[ii, (2 * C + ii) // 3 - 170] = w_v
    w0[:, 86] = b_v
    w0e = np.zeros((C, 89), np.float32)
    w0e[:, 0:87] = w0
    w0e[:, 88] = w_v              # c2 column = rowsum of w0[:, 0:86]
    w0e = w0e.astype(_BF)
    w0t = np.ascontiguousarray(w0[:, 0:86].T).astype(_BF)   # [86, C]

    G1, G2 = gam.sum(), (gam * gam).sum()
    Gb = (gam * bet).sum()
    B1, B2 = bet.sum(), (bet * bet).sum()
    sc = np.zeros((1, 8), np.float32)
    sc[0, :6] = [G1, G2, Gb, B1, -Gb, -G1]

    crows = np.stack([w_k, b_k, float(S) * b_k, B1 * b_k + B2 * w_k],
                     0).astype(np.float32)

    idb = np.eye(128, dtype=np.float32).astype(_BF)

    in_maps = []
    for r in range(NCORES):
        b = r // 2
        h = r % 2
        sl = slice(h * SHH, (h + 1) * SHH)
        gl = gam[sl]
        bl = bet[sl]

        xs_bf = xf[b * C:(b + 1) * C, sl].astype(_BF)
        gx = (xf[b * C:(b + 1) * C, sl] * gl[None, :]).astype(np.float32)
        # xst partition-major, gamma-prescaled: [g*x_A | g, b, 1, 1/g | g*x_K]
        xst = np.empty((128, NST, UTW2), _BF)
        xst[:, :, 0:86] = gx[0:86, :].reshape(86, NST, 128).transpose(2, 1, 0)
        xst[:, :, 86] = gl.reshape(NST, 128).T
        xst[:, :, 87] = bl.reshape(NST, 128).T
        xst[:, :, 88] = 1.0
        xst[:, :, 89] = (1.0 / gl.astype(np.float64)).astype(_BF).reshape(
            NST, 128).T
        xst[:, :, 90:176] = gx[85:171, :].reshape(86, NST, 128).transpose(2, 1, 0)

        rhsm = np.zeros((128, SHH), np.float32)
        rhsm[0:86] = gx[170:256, :]
        rhsm[86] = 1.0
        rhsm[87] = -gl
        rhsm[88] = bl

        in_maps.append({
            "xs": xs_bf,
            "xst": xst.reshape(128, NST * UTW2),
            "rhsm": rhsm.astype(_BF),
            "ekt": ekt,
            "eqtL": eqtL,
            "w0e": w0e,
            "w0t": w0t,
            "crows": crows,
            "idb": idb,
            "sc": sc,
        })
    return in_maps


def kernel(x, gamma, beta, w_qkv, b_qkv):
    from concourse.bass_utils import run_bass_kernel_spmd

    if "nc" not in _cache:
        _cache["nc"] = _build_program()
    nc = _cache["nc"]

    in_maps = _host_prep(x, gamma, beta, w_qkv, b_qkv)
    res = run_bass_kernel_spmd(nc, in_maps, core_ids=list(range(NCORES)))
    out = np.empty((B * C, S), np.float32)
    for r in range(NCORES):
        b = r // 2
        h = r % 2
        out[b * C:(b + 1) * C, h * SHH:(h + 1) * SHH] = res.results[r]["out"]
    return out.reshape(np.asarray(x).shape)


if __name__ == "__main__":
    rng = np.random.default_rng(0)
    inputs = {
        "x": rng.standard_normal((B, C, 32, 32, 32)).astype(np.float32),
        "gamma": (1 + 0.1 * rng.standard_normal((32, 32, 32))).astype(np.float32),
        "beta": (0.1 * rng.standard_normal((32, 32, 32))).astype(np.float32),
        "w_qkv": (0.5 * rng.standard_normal(3 * C)).astype(np.float32),
        "b_qkv": (0.05 * rng.standard_normal(3 * C)).astype(np.float32),
    }
    o = kernel(**inputs)
    print("out", o.shape, o.dtype, float(np.abs(o).mean()))
